# revision 2
# baseline (speedup 1.0000x reference)
"""AuxLossFreeMoE TRN2 kernel: 16-expert top-2 sigmoid-gated MoE + shared expert.

Strategy (8 NeuronCores, one SPMD Bass program, per-core data via inputs):
  - Routing (sigmoid gating + top-2 + weight normalization) runs on host with
    the exact jax CPU ops of the reference (ties in the saturated sigmoid make
    device routing numerically fragile; routing is 0.13% of total FLOPs).
  - Expert-parallel routed FFN in bf16 with static load balancing: each core
    gets three "pieces" of capacity [768, 384, 128] token-slots. Expert token
    lists are carved into these pieces; hot experts are split across cores
    with their tokens dealt capacity-proportionally (_stride_split_experts) so
    every piece's contributions spread evenly over all owner cores, minimizing
    the AllToAll bucket capacity. bf16 keeps matmuls at the full PE rate while
    halving weight/token DMA vs fp32; routed-path quantization error is ~4e-3
    rms of the output, well inside the 2e-2 gate.
  - The host pre-gathers and pre-transposes each core's tokens (dispatch-side
    sharding), so the device does pure dense SwiGLU; rows are scaled by the
    combine weight on PSUM eviction and scattered (bf16) into the
    owner-bucketed send buffer.
  - One bf16 AllToAll moves contributions to token-owner cores, overlapped by
    the shared-expert phase.
  - Shared expert runs on each core's 512 owned tokens in fp8 (e4m3) using
    DoubleRow perf-mode matmuls (2x PE throughput, 256-deep contraction per
    instruction). Weights are pre-scaled by 64 and x pre-quantized on the
    host; scales are folded into activation evictions. The shared output is
    multiplied by ratio=0.1, which also scales the fp8 quantization error to
    ~1.2e-2 max relative -- inside the gate with margin.
  - Shared down-projection runs token-tile-major so the combine (gather two
    bf16 contributions per token + add shared + write fp32 output) overlaps
    the remaining shared matmuls.
"""

import os
import numpy as np
import ml_dtypes

B, S, H = 4, 1024, 2048
E = 16
TOPK = 2
I = 1024
ISH = 2048
RATIO = 0.1
EPS = 1e-9
T = B * S
NC = 8
P = 128
TOWN = T // NC  # 512 tokens owned per core
PIECE_SIZES = (768, 384, 128)
CAP = sum(PIECE_SIZES)  # 1280 slots per core
N_TILES = CAP // P  # 10
KC_H = H // P    # 16
M_I = I // P     # 8
M_ISH = ISH // P  # 16
KC2 = H // 256   # 8 double-row contraction chunks over H
IC2 = ISH // 256  # 8 double-row contraction chunks over ISH
HB = H // 512    # 4 output column blocks
SW = 64.0        # fp8 weight scale (shared expert)
SH = 4.0         # fp8 h scale (shared expert)
DUMMY_TOK = T  # extra zero row in x_pad
BIG = 10 ** 9

BF16 = ml_dtypes.bfloat16
F8 = ml_dtypes.float8_e4m3

_COMPILED = {}
SKIP_PHASES = frozenset()  # debug: subsets of {'shared','routed','a2a','combine'}


def _enable_jax_cache():
    import jax
    try:
        cache_dir = os.environ.get("KERNEL_JAX_CACHE", "/tmp/jax_moe_cache")
        jax.config.update("jax_compilation_cache_dir", cache_dir)
        jax.config.update("jax_persistent_cache_min_compile_time_secs", 0.0)
    except Exception:
        pass


def _host_routing(x, centroids, gate_bias):
    """Bit-identical routing to the reference (jax CPU ops)."""
    import jax
    import jax.numpy as jnp
    cpu = jax.devices("cpu")[0]
    with jax.default_device(cpu):
        xj = jax.device_put(np.asarray(x), cpu)
        cj = jax.device_put(np.asarray(centroids), cpu)
        gj = jax.device_put(np.asarray(gate_bias), cpu)
        aff = jax.nn.sigmoid(jnp.einsum('bsh,eh->bse', xj, cj))
        biased = aff + gj
        _, top_idx = jax.lax.top_k(biased, TOPK)
        top_aff = jnp.take_along_axis(aff, top_idx, axis=-1)
        weights = top_aff / (top_aff.sum(-1, keepdims=True) + EPS)
    top_idx = np.asarray(top_idx).reshape(T, TOPK).astype(np.int64)
    weights = np.asarray(weights).reshape(T, TOPK).astype(np.float32)
    return top_idx, weights


def _assign_pieces(counts):
    """Carve expert token lists into pieces of sizes 768/384/128 (8 of each),
    then pack one piece of each size per core, co-locating same-expert pieces
    to minimize weight traffic. Returns per-core piece lists
    [(expert, offset_in_expert_list, realcount, size), ...] ordered [A,B,C]."""
    avail = {768: 8, 384: 8, 128: 8}
    pieces = {768: [], 384: [], 128: []}
    order = np.argsort(-np.asarray(counts), kind="stable")
    for e in order:
        rem = int(counts[e])
        off = 0
        if rem == 0:
            continue
        while rem > 0:
            if rem > 384 and avail[768] > 0:
                sz = 768
            elif rem > 128 and avail[384] > 0:
                sz = 384
            elif rem <= 128 and avail[128] > 0:
                sz = 128
            elif avail[384] > 0:
                sz = 384
            elif avail[768] > 0:
                sz = 768
            else:
                raise RuntimeError("piece inventory exhausted; routing distribution unexpected")
            avail[sz] -= 1
            take = min(rem, sz)
            pieces[sz].append((int(e), off, take, sz))
            off += take
            rem -= take
    # dummy pieces for unused inventory
    for sz in (768, 384, 128):
        while avail[sz] > 0:
            pieces[sz].append((0, 0, 0, sz))
            avail[sz] -= 1
    # pack cores: one piece of each size; prefer same-expert grouping
    cores = []
    used_b = [False] * 8
    used_c = [False] * 8
    for a in pieces[768]:
        grp = [a]
        be = next((j for j, bp in enumerate(pieces[384])
                   if not used_b[j] and bp[2] > 0 and bp[0] == a[0]), None)
        if be is None:
            be = next(j for j, _ in enumerate(pieces[384]) if not used_b[j])
        used_b[be] = True
        grp.append(pieces[384][be])
        exps = {a[0], pieces[384][be][0]}
        ce = next((j for j, cp in enumerate(pieces[128])
                   if not used_c[j] and cp[2] > 0 and cp[0] in exps), None)
        if ce is None:
            ce = next(j for j, _ in enumerate(pieces[128]) if not used_c[j])
        used_c[ce] = True
        grp.append(pieces[128][ce])
        cores.append(grp)
    return cores



def _optimize_pairing(cores, lists):
    """Re-pair B/C pieces across cores to flatten the per-(core, owner)
    contribution-count maxima, which sets the AllToAll bucket capacity."""
    import itertools

    def owner_vec(piece):
        e, off, cnt, sz = piece
        v = np.zeros(NC, np.int64)
        for j in range(cnt):
            v[lists[e][off + j] // TOWN] += 1
        return v

    def max_bucket(groups):
        return max(int(sum((owner_vec(p) for p in g), np.zeros(NC, np.int64)).max())
                   for g in groups)

    A = [g[0] for g in cores]
    Bp = [g[1] for g in cores]
    Cp = [g[2] for g in cores]
    Av = [owner_vec(p) for p in A]
    Bv = [owner_vec(p) for p in Bp]
    Cv = [owner_vec(p) for p in Cp]
    orderA = sorted(range(NC), key=lambda i: -Av[i].max())
    availB = list(range(NC))
    assignB = {}
    for i in orderA:
        j = min(availB, key=lambda j: (Av[i] + Bv[j]).max())
        assignB[i] = j
        availB.remove(j)
    mid = {i: Av[i] + Bv[assignB[i]] for i in range(NC)}
    orderA2 = sorted(range(NC), key=lambda i: -mid[i].max())
    availC = list(range(NC))
    assignC = {}
    for i in orderA2:
        j = min(availC, key=lambda j: (mid[i] + Cv[j]).max())
        assignC[i] = j
        availC.remove(j)
    best = [[A[i], Bp[assignB[i]], Cp[assignC[i]]] for i in range(NC)]
    for _ in range(50):
        improved = False
        for (i, j) in itertools.combinations(range(NC), 2):
            for slot in (1, 2):
                g = [list(x) for x in best]
                g[i][slot], g[j][slot] = g[j][slot], g[i][slot]
                if max_bucket(g) < max_bucket(best):
                    best = g
                    improved = True
        if not improved:
            break
    return [tuple(g) for g in best] if max_bucket(best) < max_bucket(cores) else cores



def _stride_split_experts(cores, lists, wvals):
    """For experts split across multiple pieces, deal their tokens to the
    pieces capacity-proportionally instead of contiguously. Token order equals
    owner order, so contiguous carving concentrates each piece's contributions
    on few owners and inflates the AllToAll bucket capacity; dealing spreads
    every piece across all owners."""
    by_expert = {}
    for c in range(len(cores)):
        for pi, (e, off, cnt, sz) in enumerate(cores[c]):
            if cnt > 0:
                by_expert.setdefault(e, []).append((c, pi, off, cnt, sz))
    new_cores = [list(g) for g in cores]
    for e, ps in by_expert.items():
        if len(ps) < 2:
            continue
        ps.sort(key=lambda t: t[2])  # original carve order by offset
        n = sum(cnt for (_, _, _, cnt, _) in ps)
        caps = [sz for (_, _, _, _, sz) in ps]
        fills = [0] * len(ps)
        buckets = [[] for _ in ps]
        for j in range(n):
            k = min((i for i in range(len(ps)) if fills[i] < caps[i]),
                    key=lambda i: fills[i] / caps[i])
            buckets[k].append(j)
            fills[k] += 1
        perm = [j for b in buckets for j in b]
        lists[e] = [lists[e][j] for j in perm]
        wvals[e] = [wvals[e][j] for j in perm]
        off = 0
        for i, (c, pi, _, _, sz) in enumerate(ps):
            new_cores[c][pi] = (e, off, fills[i], sz)
            off += fills[i]
    return [tuple(g) for g in new_cores], lists, wvals


def _build_program(scap, n_reps=1):
    """Build the SPMD Bass program (same for all cores)."""
    import concourse.bass as bass
    import concourse.mybir as mybir
    import concourse.tile as tile
    from concourse import bacc
    from concourse.masks import make_identity

    dt = mybir.dt
    AF = mybir.ActivationFunctionType
    ALU = mybir.AluOpType
    DR = mybir.MatmulPerfMode.DoubleRow

    SCAP = scap
    SEND_ROWS = NC * SCAP

    nc = bacc.Bacc("TRN2", target_bir_lowering=False, num_devices=NC)

    f32, bf16, f8, i32 = dt.float32, dt.bfloat16, dt.float8e4, dt.int32

    xg_in = nc.dram_tensor("xg_in", [KC_H, P, CAP], bf16, kind="ExternalInput")
    wslot = nc.dram_tensor("wslot", [N_TILES, P], f32, kind="ExternalInput")
    send_pos = nc.dram_tensor("send_pos", [N_TILES, P], i32, kind="ExternalInput")
    recv_idx = nc.dram_tensor("recv_idx", [2, TOWN // P, P], i32, kind="ExternalInput")
    wg_in = nc.dram_tensor("wg_in", [3, M_I, P, KC_H, P], bf16, kind="ExternalInput")
    wu_in = nc.dram_tensor("wu_in", [3, M_I, P, KC_H, P], bf16, kind="ExternalInput")
    wd_in = nc.dram_tensor("wd_in", [3, M_I, P, H], bf16, kind="ExternalInput")
    wgs_in = nc.dram_tensor("wgs_in", [M_ISH, P, KC2, 2, P], f8, kind="ExternalInput")
    wus_in = nc.dram_tensor("wus_in", [M_ISH, P, KC2, 2, P], f8, kind="ExternalInput")
    wds_in = nc.dram_tensor("wds_in", [P, HB, IC2, 2, 512], f8, kind="ExternalInput")
    xq_in = nc.dram_tensor("xq_in", [P, KC2, 2, TOWN], f8, kind="ExternalInput")

    out_own = nc.dram_tensor("out_own", [TOWN, H], f32, kind="ExternalOutput")

    send_buf = nc.dram_tensor("send_buf", [SEND_ROWS, H], bf16)
    recv_buf = nc.dram_tensor("recv_buf", [SEND_ROWS, H], bf16)

    # piece -> (local tile offset, number of slot tiles, matmul blocks)
    piece_tiles = [sz // P for sz in PIECE_SIZES]
    piece_tile_off = [0, 6, 9]
    piece_blocks = {0: [(0, 512), (512, 256)], 1: [(0, 384)], 2: [(0, 128)]}

    with tile.TileContext(nc) as tc:
      with (
          tc.tile_pool(name="const", bufs=1) as constp,
          tc.tile_pool(name="big", bufs=1) as bigp,
          tc.tile_pool(name="io", bufs=2) as iop,
      ):
        for _rep in range(n_reps):
            shared_tok = bigp.tile([P, TOWN // P, H], bf16, name="shared_tok",
                                   tag="shared_tok")

            # ---------------- routed experts: 3 pieces ----------------
            if "routed" not in SKIP_PHASES:
              with (
                  tc.tile_pool(name="rtbig", bufs=1) as rtbig,
                  tc.tile_pool(name="rtw", bufs=2) as rtw,
                  tc.tile_pool(name="rtwork", bufs=2) as work,
              ):
                  for p_i in range(3):
                      n_t = piece_tiles[p_i]
                      t_off = piece_tile_off[p_i]

                      up_ps = tc.tile_pool(name=f"upps{p_i}", bufs=1, space="PSUM")
                      psp = up_ps.__enter__()

                      # load pre-gathered, pre-transposed tokens for this piece
                      xgT = rtbig.tile([P, KC_H, 768], bf16, name="xgT", tag="xgT")
                      sz_p = PIECE_SIZES[p_i]
                      for kc in range(KC_H):
                          nc.sync.dma_start(
                              xgT[:, kc, :sz_p],
                              xg_in[kc, :, t_off * P:t_off * P + sz_p])
                      wts = []
                      sidx = []
                      for st in range(n_t):
                          w_t = constp.tile([P, 1], f32, name=f"w_t{p_i}_{st}", tag=f"w_t{t_off + st}")
                          nc.sync.dma_start(w_t[:], wslot[t_off + st][:, None])
                          wts.append(w_t)
                          si_t = constp.tile([P, 1], i32, name=f"si_t{p_i}_{st}", tag=f"si_t{t_off + st}")
                          nc.sync.dma_start(si_t[:], send_pos[t_off + st][:, None])
                          sidx.append(si_t)

                      # up/gate projections -> h [i, slots] bf16
                      h = rtbig.tile([P, M_I, 768], bf16, name="h", tag="h")
                      for m in range(M_I):
                          wg_t = rtw.tile([P, KC_H, P], bf16, name="wg_t", tag="wg_t")
                          wu_t = rtw.tile([P, KC_H, P], bf16, name="wu_t", tag="wu_t")
                          if "wdma" not in SKIP_PHASES:
                              nc.sync.dma_start(wg_t[:], wg_in[p_i, m])
                              nc.sync.dma_start(wu_t[:], wu_in[p_i, m])
                          for (b0, bn) in piece_blocks[p_i]:
                              if "mm" in SKIP_PHASES:
                                  continue
                              psg2 = psp.tile([P, 512], f32, name="psg2", tag="psg", bufs=2)
                              psu2 = psp.tile([P, 512], f32, name="psu2", tag="psu", bufs=2)
                              for kc in range(KC_H):
                                  nc.tensor.matmul(psg2[:, :bn], wg_t[:, kc, :],
                                                   xgT[:, kc, b0:b0 + bn],
                                                   start=(kc == 0), stop=(kc == KC_H - 1))
                              for kc in range(KC_H):
                                  nc.tensor.matmul(psu2[:, :bn], wu_t[:, kc, :],
                                                   xgT[:, kc, b0:b0 + bn],
                                                   start=(kc == 0), stop=(kc == KC_H - 1))
                              sg2 = work.tile([P, 512], bf16, name="sg2", tag="sg2")
                              nc.scalar.activation(sg2[:, :bn], psg2[:, :bn], AF.Silu)
                              nc.vector.tensor_mul(h[:, m, b0:b0 + bn], sg2[:, :bn], psu2[:, :bn])

                      # down projection, token-major out; scale; scatter to send_buf
                      up_ps.__exit__(None, None, None)
                      dn_ps = tc.tile_pool(name=f"dnps{p_i}", bufs=1, space="PSUM")
                      dpsp = dn_ps.__enter__()
                      y_tok = [rtbig.tile([P, H], bf16, name=f"y_tok{st}", tag=f"y_tok{st}")
                               for st in range(n_t)]
                      n_hb = H // 512
                      for hb in range(n_hb):
                          ps_d = [dpsp.tile([P, 512], f32, name=f"ps_d{st}", tag=f"ps_d{st}")
                                  for st in range(n_t)]
                          for ic in range(M_I):
                              wd_t = rtw.tile([P, 512], bf16, name="wd_t", tag="wd_t", bufs=4)
                              if "wdma" not in SKIP_PHASES:
                                  nc.sync.dma_start(wd_t[:], wd_in[p_i, ic][:, hb * 512:(hb + 1) * 512])
                              for st in range(n_t):
                                  nc.tensor.matmul(ps_d[st][:], h[:, ic, st * P:(st + 1) * P],
                                                   wd_t[:], start=(ic == 0), stop=(ic == M_I - 1))
                          for st in range(n_t):
                              nc.vector.tensor_scalar_mul(
                                  y_tok[st][:, hb * 512:(hb + 1) * 512],
                                  ps_d[st][:], wts[st][:, :1])
                      for st in range(n_t):
                          if "scatter" in SKIP_PHASES:
                              continue
                          nc.gpsimd.indirect_dma_start(
                              out=send_buf[:, :], in_=y_tok[st][:],
                              out_offset=bass.IndirectOffsetOnAxis(ap=sidx[st][:, :1], axis=0),
                              in_offset=None,
                              bounds_check=SEND_ROWS - 1,
                              oob_is_err=False)
                      dn_ps.__exit__(None, None, None)

            # ---------------- all-to-all combine ----------------
            if "a2a" not in SKIP_PHASES:
              nc.gpsimd.collective_compute(
                "AllToAll",
                mybir.AluOpType.bypass,
                replica_groups=[list(range(NC))],
                ins=[send_buf[:, :].opt()],
                outs=[recv_buf[:, :].opt()],
              )

            # ---------------- shared expert (own 512 tokens, fp8 DoubleRow) --
            if "shared" in SKIP_PHASES:
                nc.vector.memset(shared_tok[:], 0.0)
                shb = None
            else:
              with (
                  tc.tile_pool(name="shbig", bufs=1) as shbig,
                  tc.tile_pool(name="shw", bufs=2) as shw,
                  tc.tile_pool(name="shps", bufs=1, space="PSUM") as psp,
              ):
                  xq = shbig.tile([P, KC2, 2, TOWN], f8, name="xq", tag="xq")
                  nc.sync.dma_start(xq[:], xq_in[:])
                  # all down-proj weights resident (32KB/partition in fp8)
                  wds_t = shbig.tile([P, HB, IC2, 2, 512], f8, name="wds_t", tag="wds_t")
                  nc.sync.dma_start(wds_t[:], wds_in[:])

                  h8 = shbig.tile([P, IC2, 2, TOWN], f8, name="h8", tag="h8")
                  for m in range(M_ISH):
                      wgs_t = shw.tile([P, KC2, 2, P], f8, name="wgs_t", tag="wgs_t", bufs=4)
                      wus_t = shw.tile([P, KC2, 2, P], f8, name="wus_t", tag="wus_t", bufs=4)
                      nc.sync.dma_start(wgs_t[:], wgs_in[m])
                      nc.sync.dma_start(wus_t[:], wus_in[m])
                      psg = psp.tile([P, TOWN], f32, name="psg", tag="psg", bufs=2)
                      psu = psp.tile([P, TOWN], f32, name="psu", tag="psu", bufs=2)
                      for kc in range(KC2):
                          nc.tensor.matmul(psg[:], wgs_t[:, kc, :, :], xq[:, kc, :, :],
                                           start=(kc == 0), stop=(kc == KC2 - 1),
                                           perf_mode=DR)
                      for kc in range(KC2):
                          nc.tensor.matmul(psu[:], wus_t[:, kc, :, :], xq[:, kc, :, :],
                                           start=(kc == 0), stop=(kc == KC2 - 1),
                                           perf_mode=DR)
                      sg = shw.tile([P, TOWN], bf16, name="sg", tag="sg")
                      su = shw.tile([P, TOWN], bf16, name="su", tag="su")
                      nc.scalar.activation(sg[:], psg[:], AF.Silu, scale=1.0 / SW)
                      nc.scalar.activation(su[:], psu[:], AF.Copy, scale=SH / SW)
                      nc.vector.tensor_mul(h8[:, m // 2, m % 2, :], sg[:], su[:])

                  # shared down-projection, token-tile-major so combine overlaps
                  with tc.tile_pool(name="cmb", bufs=2) as cmb:
                      for tt in range(TOWN // P):
                          i1 = iop.tile([P, 1], i32, name="i1", tag="i1")
                          i2 = iop.tile([P, 1], i32, name="i2", tag="i2")
                          nc.sync.dma_start(i1[:], recv_idx[0, tt][:, None])
                          nc.sync.dma_start(i2[:], recv_idx[1, tt][:, None])
                          g1 = cmb.tile([P, H], bf16, name="g1", tag="g1")
                          g2 = cmb.tile([P, H], bf16, name="g2", tag="g2")
                          if "a2a" not in SKIP_PHASES:
                              nc.gpsimd.indirect_dma_start(
                                  out=g1[:], out_offset=None, in_=recv_buf[:, :],
                                  in_offset=bass.IndirectOffsetOnAxis(ap=i1[:, :1], axis=0))
                              nc.gpsimd.indirect_dma_start(
                                  out=g2[:], out_offset=None, in_=recv_buf[:, :],
                                  in_offset=bass.IndirectOffsetOnAxis(ap=i2[:, :1], axis=0))
                          else:
                              nc.vector.memset(g1[:], 0.0)
                              nc.vector.memset(g2[:], 0.0)
                          acc = cmb.tile([P, H], f32, name="acc", tag="acc")
                          for hb in range(HB):
                              ps_sh = psp.tile([P, 512], f32, name="ps_sh",
                                               tag="ps_sh", bufs=2)
                              for ic in range(IC2):
                                  nc.tensor.matmul(ps_sh[:], h8[:, ic, :, tt * P:(tt + 1) * P],
                                                   wds_t[:, hb, ic, :, :],
                                                   start=(ic == 0), stop=(ic == IC2 - 1),
                                                   perf_mode=DR)
                              nc.scalar.activation(
                                  shared_tok[:, tt, hb * 512:(hb + 1) * 512],
                                  ps_sh[:], AF.Copy, scale=RATIO / (SH * SW))
                          nc.vector.tensor_add(acc[:], g1[:], g2[:])
                          nc.vector.tensor_add(acc[:], acc[:], shared_tok[:, tt, :])
                          nc.sync.dma_start(out_own[tt * P:(tt + 1) * P, :], acc[:])

            if "shared" in SKIP_PHASES:
              # fallback combine without shared overlap (debug only)
              with tc.tile_pool(name="cmb", bufs=2) as cmb:
                for tt in range(TOWN // P):
                    i1 = iop.tile([P, 1], i32, name="i1", tag="i1")
                    i2 = iop.tile([P, 1], i32, name="i2", tag="i2")
                    nc.sync.dma_start(i1[:], recv_idx[0, tt][:, None])
                    nc.sync.dma_start(i2[:], recv_idx[1, tt][:, None])
                    g1 = cmb.tile([P, H], bf16, name="g1", tag="g1")
                    g2 = cmb.tile([P, H], bf16, name="g2", tag="g2")
                    nc.gpsimd.indirect_dma_start(
                        out=g1[:], out_offset=None, in_=recv_buf[:, :],
                        in_offset=bass.IndirectOffsetOnAxis(ap=i1[:, :1], axis=0))
                    nc.gpsimd.indirect_dma_start(
                        out=g2[:], out_offset=None, in_=recv_buf[:, :],
                        in_offset=bass.IndirectOffsetOnAxis(ap=i2[:, :1], axis=0))
                    acc = cmb.tile([P, H], f32, name="acc", tag="acc")
                    nc.vector.tensor_add(acc[:], g1[:], g2[:])
                    nc.vector.tensor_add(acc[:], acc[:], shared_tok[:, tt, :])
                    nc.sync.dma_start(out_own[tt * P:(tt + 1) * P, :], acc[:])

    nc.finalize()
    return nc


def prepare_in_maps(x, centroids, gate_bias, wg_s, wu_s, wd_s, wg, wu, wd):
    x = np.ascontiguousarray(np.asarray(x, dtype=np.float32))
    wg = np.asarray(wg, dtype=np.float32)
    wu = np.asarray(wu, dtype=np.float32)
    wd = np.asarray(wd, dtype=np.float32)

    top_idx, weights = _host_routing(x, centroids, gate_bias)

    # expert token lists in token order
    lists = [[] for _ in range(E)]
    wvals = [[] for _ in range(E)]
    for t in range(T):
        for k in range(TOPK):
            e = int(top_idx[t, k])
            lists[e].append(t)
            wvals[e].append(weights[t, k])
    counts = [len(l) for l in lists]
    cores = _assign_pieces(counts)
    cores, lists, wvals = _stride_split_experts(cores, lists, wvals)
    cores = _optimize_pairing(cores, lists)

    # per-core slot tables
    tok_ids = np.full((NC, N_TILES, P), DUMMY_TOK, dtype=np.int32)
    wslot = np.zeros((NC, N_TILES, P), dtype=np.float32)
    piece_expert = np.zeros((NC, 3), dtype=np.int64)
    for c in range(NC):
        loc = 0
        for pi, (e, off, cnt, sz) in enumerate(cores[c]):
            piece_expert[c, pi] = e
            pts = [(lists[e][off + j], wvals[e][off + j]) for j in range(cnt)]
            pts.sort(key=lambda tw: (tw[0] // TOWN, tw[0]))
            for j, (t, w) in enumerate(pts):
                tok_ids[c, (loc + j) // P, (loc + j) % P] = t
                wslot[c, (loc + j) // P, (loc + j) % P] = w
            loc += sz

    # send positions / recv indices
    cnt_co = np.zeros((NC, NC), dtype=np.int64)
    contrib = [[] for _ in range(T)]  # (core, pos) per contribution
    for c in range(NC):
        for loc in range(CAP):
            t = int(tok_ids[c, loc // P, loc % P])
            if t == DUMMY_TOK:
                continue
            o = t // TOWN
            pos = cnt_co[c, o]
            cnt_co[c, o] += 1
            contrib[t].append((c, int(pos)))
    SCAP = int(((cnt_co.max() + 15) // 16) * 16)
    # destination row = owner * SCAP + pos
    send_pos_arr = np.full((NC, N_TILES, P), BIG, dtype=np.int32)
    cnt_co2 = np.zeros((NC, NC), dtype=np.int64)
    for c in range(NC):
        for loc in range(CAP):
            t = int(tok_ids[c, loc // P, loc % P])
            if t == DUMMY_TOK:
                continue
            o = t // TOWN
            pos = cnt_co2[c, o]
            cnt_co2[c, o] += 1
            send_pos_arr[c, loc // P, loc % P] = o * SCAP + pos

    recv_idx = np.zeros((NC, 2, TOWN // P, P), dtype=np.int32)
    for t in range(T):
        o = t // TOWN
        tl = t % TOWN
        assert len(contrib[t]) == 2, (t, contrib[t])
        for k, (c, pos) in enumerate(contrib[t]):
            recv_idx[o, k, tl // P, tl % P] = c * SCAP + pos

    # weight tensors, matmul-ready tiling
    def tile_up(w2d, mm):  # [H, mm*128] -> [mm, 128, KC_H, 128]
        return np.ascontiguousarray(
            w2d.reshape(KC_H, P, mm, P).transpose(2, 1, 0, 3))

    def tile_dn(w2d, mm):  # [mm*128, H] -> [mm, 128, H]
        return np.ascontiguousarray(w2d.reshape(mm, P, H))

    wg_t = np.zeros((NC, 3, M_I, P, KC_H, P), dtype=BF16)
    wu_t = np.zeros((NC, 3, M_I, P, KC_H, P), dtype=BF16)
    wd_t = np.zeros((NC, 3, M_I, P, H), dtype=BF16)
    done = {}
    for c in range(NC):
        for pi, (e, off, cnt, sz) in enumerate(cores[c]):
            if cnt == 0:
                continue
            if e not in done:
                done[e] = (tile_up(wg[e], M_I).astype(BF16),
                           tile_up(wu[e], M_I).astype(BF16),
                           tile_dn(wd[e], M_I).astype(BF16))
            wg_t[c, pi], wu_t[c, pi], wd_t[c, pi] = done[e]

    # shared-expert fp8 tensors (scaled), DoubleRow layouts
    def tile_up8(w2d):  # [H, ISH] -> [M_ISH, P, KC2, 2, P]
        w = (np.asarray(w2d, np.float32) * SW).reshape(KC2, 2, P, M_ISH, P)
        return np.ascontiguousarray(w.transpose(3, 2, 0, 1, 4)).astype(F8)

    def tile_dn8(w2d):  # [ISH, H] -> [P, HB, IC2, 2, 512]
        w = (np.asarray(w2d, np.float32) * SW).reshape(IC2, 2, P, HB, 512)
        return np.ascontiguousarray(w.transpose(2, 3, 0, 1, 4)).astype(F8)

    wgs_t = tile_up8(wg_s)
    wus_t = tile_up8(wu_s)
    wds_t = tile_dn8(wd_s)

    x_flat = x.reshape(T, H)
    x_pad = np.vstack([x_flat, np.zeros((1, H), np.float32)])

    in_maps = []
    for c in range(NC):
        xo = x_flat[c * TOWN:(c + 1) * TOWN]  # [TOWN, H]
        # [P, KC2, 2, TOWN] with h = kc2*256 + j*128 + p
        xq = np.ascontiguousarray(
            xo.T.reshape(KC2, 2, P, TOWN).transpose(2, 0, 1, 3)).astype(F8)
        xg_c = np.ascontiguousarray(
            x_pad[tok_ids[c].reshape(-1)].T.reshape(KC_H, P, CAP)).astype(BF16)
        in_maps.append({
            "xg_in": xg_c,
            "wslot": wslot[c],
            "send_pos": send_pos_arr[c],
            "recv_idx": recv_idx[c],
            "wg_in": wg_t[c],
            "wu_in": wu_t[c],
            "wd_in": wd_t[c],
            "wgs_in": wgs_t,
            "wus_in": wus_t,
            "wds_in": wds_t,
            "xq_in": xq,
        })

    return in_maps, SCAP


def get_program(scap, n_reps=1):
    key = ("moe", scap, n_reps)
    if key not in _COMPILED:
        _COMPILED[key] = _build_program(scap, n_reps)
    return _COMPILED[key]


_RUNNER = {}


def _build_runner(nc, n_cores=NC):
    """Build a reusable PJRT executable for the finalized Bass program.
    Mirrors concourse.bass2jax.run_bass_via_pjrt but without output donation,
    so the jitted callable can be invoked repeatedly and its HLO is stable
    across processes (persistent-cache friendly)."""
    import jax
    import concourse.mybir as mybir
    from concourse import bass2jax as b2j
    from jax.experimental.shard_map import shard_map
    from jax.sharding import Mesh, PartitionSpec, NamedSharding

    b2j.install_neuronx_cc_hook()
    partition_name = nc.partition_id_tensor.name if nc.partition_id_tensor else None
    in_names, out_names, out_avals, zero_outs = [], [], [], []
    for alloc in nc.m.functions[0].allocations:
        if not isinstance(alloc, mybir.MemoryLocationSet):
            continue
        name = alloc.memorylocations[0].name
        if alloc.kind == "ExternalInput":
            if name != partition_name:
                in_names.append(name)
        elif alloc.kind == "ExternalOutput":
            shape = tuple(alloc.tensor_shape)
            dtype = mybir.dt.np(alloc.dtype)
            out_avals.append(jax.core.ShapedArray(shape, dtype))
            out_names.append(name)
            zero_outs.append(np.zeros(shape, dtype))
    n_params = len(in_names)
    all_in_names = in_names + out_names
    if partition_name is not None:
        all_in_names = all_in_names + [partition_name]

    def _body(*args):
        operands = list(args)
        if partition_name is not None:
            operands.append(b2j.partition_id_tensor())
        outs = b2j._bass_exec_p.bind(
            *operands,
            out_avals=tuple(out_avals),
            in_names=tuple(all_in_names),
            out_names=tuple(out_names),
            lowering_input_output_aliases=(),
            sim_require_finite=True,
            sim_require_nnan=True,
            nc=nc,
        )
        return tuple(outs)

    devices = jax.devices()[:n_cores]
    mesh = Mesh(np.asarray(devices), ("core",))
    spec = PartitionSpec("core")
    sharded = jax.jit(
        shard_map(_body, mesh=mesh, in_specs=(spec,) * (n_params + len(out_names)),
                  out_specs=(spec,) * len(out_names), check_rep=False),
        keep_unused=True,
    )
    sh = NamedSharding(mesh, spec)

    def run(in_maps):
        concat_in = [
            np.concatenate([np.asarray(in_maps[c][nm]) for c in range(n_cores)], axis=0)
            for nm in in_names
        ]
        concat_zeros = [np.zeros((n_cores * z.shape[0], *z.shape[1:]), z.dtype)
                        for z in zero_outs]
        dev_in = [jax.device_put(a, sh) for a in concat_in]
        dev_zero = [jax.device_put(a, sh) for a in concat_zeros]
        out = sharded(*dev_in, *dev_zero)
        jax.block_until_ready(out)
        return ({nm: np.asarray(out[i]) for i, nm in enumerate(out_names)},
                (sharded, dev_in, dev_zero))

    return run


def kernel(x, centroids, gate_bias, wg_s, wu_s, wd_s, wg, wu, wd):
    _enable_jax_cache()
    in_maps, scap = prepare_in_maps(x, centroids, gate_bias, wg_s, wu_s, wd_s, wg, wu, wd)
    nc = get_program(scap)
    key = ("run", scap)
    if key not in _RUNNER:
        _RUNNER[key] = _build_runner(nc)
    outs, _ = _RUNNER[key](in_maps)
    out = outs["out_own"].reshape(NC, TOWN, H)
    return np.ascontiguousarray(out.reshape(B, S, H))


# revision 10
# speedup vs baseline: 1.8949x; 1.8949x over previous
"""AuxLossFreeMoE TRN2 kernel: 16-expert top-2 sigmoid-gated MoE + shared expert.

Strategy (8 NeuronCores, one SPMD Bass program, per-core data via inputs):
  - Routing (sigmoid gating + top-2 + weight normalization) runs on host with
    the exact jax CPU ops of the reference (ties in the saturated sigmoid make
    device routing numerically fragile; routing is 0.13% of total FLOPs).
  - Expert-parallel routed FFN in bf16 with static load balancing: each core
    gets three "pieces" of capacity [768, 384, 128] token-slots. Expert token
    lists are carved into these pieces; hot experts are split across cores
    with their tokens dealt capacity-proportionally (_stride_split_experts) so
    every piece's contributions spread evenly over all owner cores, minimizing
    the AllToAll bucket capacity. bf16 keeps matmuls at the full PE rate while
    halving weight/token DMA vs fp32; routed-path quantization error is ~4e-3
    rms of the output, well inside the 2e-2 gate.
  - The host pre-gathers and pre-transposes each core's tokens (dispatch-side
    sharding), so the device does pure dense SwiGLU; rows are scaled by the
    combine weight on PSUM eviction and scattered (bf16) into the
    owner-bucketed send buffer.
  - One bf16 AllToAll moves contributions to token-owner cores, overlapped by
    the shared-expert phase.
  - Shared expert runs on each core's 512 owned tokens in fp8 (e4m3) using
    DoubleRow perf-mode matmuls (2x PE throughput, 256-deep contraction per
    instruction). Weights are pre-scaled by 64 and x pre-quantized on the
    host; scales are folded into activation evictions. The shared output is
    multiplied by ratio=0.1, which also scales the fp8 quantization error to
    ~1.2e-2 max relative -- inside the gate with margin.
  - Shared down-projection runs token-tile-major so the combine (gather two
    bf16 contributions per token + add shared + write fp32 output) overlaps
    the remaining shared matmuls.
"""

import os
import numpy as np
import ml_dtypes

B, S, H = 4, 1024, 2048
E = 16
TOPK = 2
I = 1024
ISH = 2048
RATIO = 0.1
EPS = 1e-9
T = B * S
NC = 8
P = 128
TOWN = T // NC  # 512 tokens owned per core
SEG_SIZES = (512, 320, 192, 128)  # per-core routed segments (one expert each)
SEG_OFF = (0, 512, 832, 1024)
NSEG = len(SEG_SIZES)
CAP = sum(SEG_SIZES)  # 1152 slots per core
N_TILES = CAP // P  # 9
TILE_GROUPS = ((0, 4), (4, 9))  # down-projection PSUM groups


def _tile_subranges(t):
    "Absolute column subranges of slot-tile t split at segment boundaries."
    t0, t1 = t * P, (t + 1) * P
    out = []
    for s in range(NSEG):
        a, b = max(t0, SEG_OFF[s]), min(t1, SEG_OFF[s] + SEG_SIZES[s])
        if a < b:
            out.append((s, a, b))
    return out
KC_H = H // P    # 16
M_I = I // P     # 8
M_ISH = ISH // P  # 16
KC2 = H // 256   # 8 double-row contraction chunks over H
IC2 = ISH // 256  # 8 double-row contraction chunks over ISH
HB = H // 512    # 4 output column blocks
SW = 64.0        # fp8 weight scale (shared expert)
SH = 4.0         # fp8 h scale (shared expert)
DUMMY_TOK = T  # extra zero row in x_pad
BIG = 10 ** 9

BF16 = ml_dtypes.bfloat16
F8 = ml_dtypes.float8_e4m3

_COMPILED = {}
# debug: subsets of {'shared','routed','a2a','combine','wdma','mm','scatter'}
SKIP_PHASES = frozenset(os.environ.get("KERNEL_SKIP", "").split(",")) - {""}


def _enable_jax_cache():
    import jax
    try:
        cache_dir = os.environ.get("KERNEL_JAX_CACHE", "/tmp/jax_moe_cache")
        jax.config.update("jax_compilation_cache_dir", cache_dir)
        jax.config.update("jax_persistent_cache_min_compile_time_secs", 0.0)
    except Exception:
        pass


def _host_routing(x, centroids, gate_bias):
    """Bit-identical routing to the reference (jax CPU ops)."""
    import jax
    import jax.numpy as jnp
    cpu = jax.devices("cpu")[0]
    with jax.default_device(cpu):
        xj = jax.device_put(np.asarray(x), cpu)
        cj = jax.device_put(np.asarray(centroids), cpu)
        gj = jax.device_put(np.asarray(gate_bias), cpu)
        aff = jax.nn.sigmoid(jnp.einsum('bsh,eh->bse', xj, cj))
        biased = aff + gj
        _, top_idx = jax.lax.top_k(biased, TOPK)
        top_aff = jnp.take_along_axis(aff, top_idx, axis=-1)
        weights = top_aff / (top_aff.sum(-1, keepdims=True) + EPS)
    top_idx = np.asarray(top_idx).reshape(T, TOPK).astype(np.int64)
    weights = np.asarray(weights).reshape(T, TOPK).astype(np.float32)
    return top_idx, weights


def _assign_segments(counts):
    """Carve expert token lists into pieces matching SEG_SIZES inventory
    (8 pieces per segment slot), single expert per piece. Returns per-core
    piece lists [(expert, offset_in_expert_list, realcount, size), ...]
    ordered by segment slot."""
    sizes = sorted(set(SEG_SIZES), reverse=True)
    avail = {s: SEG_SIZES.count(s) * NC for s in sizes}
    pieces = {s: [] for s in sizes}
    order = np.argsort(-np.asarray(counts), kind="stable")
    for e in order:
        rem = int(counts[e])
        off = 0
        if rem == 0:
            continue
        while rem > 0:
            free = [s for s in sizes if avail[s] > 0]
            if not free:
                raise RuntimeError("piece inventory exhausted; unexpected routing distribution")
            cover = [s for s in free if s >= rem]
            sz = min(cover) if cover else max(free)
            avail[sz] -= 1
            take = min(rem, sz)
            pieces[sz].append((int(e), off, take, sz))
            off += take
            rem -= take
    for sz in sizes:
        while avail[sz] > 0:
            pieces[sz].append((0, 0, 0, sz))
            avail[sz] -= 1
    # deal pieces of each size to slots that use that size
    size_slots = {}
    for sl, s in enumerate(SEG_SIZES):
        size_slots.setdefault(s, []).append(sl)
    cores = [[None] * NSEG for _ in range(NC)]
    for s in sizes:
        ps = sorted(pieces[s], key=lambda p: -p[2])
        slots = size_slots[s]
        k = 0
        for sl in slots:
            for c in range(NC):
                cores[c][sl] = ps[k]
                k += 1
    return [tuple(g) for g in cores]


def _optimize_pairing(cores, lists):
    """Re-assign non-first segment pieces across cores to flatten the
    per-(core, owner) contribution-count maxima, which set the AllToAll
    bucket capacity."""
    import itertools

    def owner_vec(piece):
        e, off, cnt, sz = piece
        v = np.zeros(NC, np.int64)
        for j in range(cnt):
            v[lists[e][off + j] // TOWN] += 1
        return v

    def max_bucket(groups):
        return max(int(sum((owner_vec(p) for p in g), np.zeros(NC, np.int64)).max())
                   for g in groups)

    best = [list(g) for g in cores]
    # greedy re-seed slot by slot: assign slot-sl pieces to cores minimizing
    # the running per-owner max
    run = [owner_vec(best[c][0]) for c in range(NC)]
    for sl in range(1, NSEG):
        pcs = [best[c][sl] for c in range(NC)]
        vecs = [owner_vec(p) for p in pcs]
        order = sorted(range(NC), key=lambda c: -run[c].max())
        availp = list(range(NC))
        for c in order:
            j = min(availp, key=lambda j: (run[c] + vecs[j]).max())
            best[c][sl] = pcs[j]
            run[c] = run[c] + vecs[j]
            availp.remove(j)
    # local-search swaps
    for _ in range(60):
        improved = False
        for (i, j) in itertools.combinations(range(NC), 2):
            for slot in range(1, NSEG):
                g = [list(x) for x in best]
                g[i][slot], g[j][slot] = g[j][slot], g[i][slot]
                if max_bucket(g) < max_bucket(best):
                    best = g
                    improved = True
        if not improved:
            break
    base = [tuple(g) for g in cores]
    return [tuple(g) for g in best] if max_bucket(best) < max_bucket(base) else base


def _stride_split_experts(cores, lists, wvals):
    """For experts split across multiple pieces, deal their tokens to the
    pieces capacity-proportionally instead of contiguously. Token order equals
    owner order, so contiguous carving concentrates each piece's contributions
    on few owners and inflates the AllToAll bucket capacity; dealing spreads
    every piece across all owners."""
    by_expert = {}
    for c in range(len(cores)):
        for pi, (e, off, cnt, sz) in enumerate(cores[c]):
            if cnt > 0:
                by_expert.setdefault(e, []).append((c, pi, off, cnt, sz))
    new_cores = [list(g) for g in cores]
    for e, ps in by_expert.items():
        if len(ps) < 2:
            continue
        ps.sort(key=lambda t: t[2])  # original carve order by offset
        n = sum(cnt for (_, _, _, cnt, _) in ps)
        caps = [sz for (_, _, _, _, sz) in ps]
        fills = [0] * len(ps)
        buckets = [[] for _ in ps]
        for j in range(n):
            k = min((i for i in range(len(ps)) if fills[i] < caps[i]),
                    key=lambda i: fills[i] / caps[i])
            buckets[k].append(j)
            fills[k] += 1
        perm = [j for b in buckets for j in b]
        lists[e] = [lists[e][j] for j in perm]
        wvals[e] = [wvals[e][j] for j in perm]
        off = 0
        for i, (c, pi, _, _, sz) in enumerate(ps):
            new_cores[c][pi] = (e, off, fills[i], sz)
            off += fills[i]
    return [tuple(g) for g in new_cores], lists, wvals


def _build_program(scap, n_reps=1):
    """Build the SPMD Bass program (same for all cores)."""
    import concourse.bass as bass
    import concourse.mybir as mybir
    import concourse.tile as tile
    from concourse import bacc
    from concourse.masks import make_identity

    dt = mybir.dt
    AF = mybir.ActivationFunctionType
    ALU = mybir.AluOpType
    DR = mybir.MatmulPerfMode.DoubleRow

    SCAP = scap
    SEND_ROWS = NC * SCAP

    nc = bacc.Bacc("TRN2", target_bir_lowering=False, num_devices=NC)

    f32, bf16, f8, i32 = dt.float32, dt.bfloat16, dt.float8e4, dt.int32

    xg_in = nc.dram_tensor("xg_in", [KC_H, P, CAP], bf16, kind="ExternalInput")
    wslot = nc.dram_tensor("wslot", [N_TILES, P], f32, kind="ExternalInput")
    send_pos = nc.dram_tensor("send_pos", [N_TILES, P], i32, kind="ExternalInput")
    recv_idx = nc.dram_tensor("recv_idx", [2, TOWN // P, P], i32, kind="ExternalInput")
    wg_in = nc.dram_tensor("wg_in", [NSEG, M_I, P, KC_H, P], bf16, kind="ExternalInput")
    wu_in = nc.dram_tensor("wu_in", [NSEG, M_I, P, KC_H, P], bf16, kind="ExternalInput")
    wd_in = nc.dram_tensor("wd_in", [NSEG, M_I, P, H], bf16, kind="ExternalInput")
    wgs_in = nc.dram_tensor("wgs_in", [M_ISH, P, KC2, 2, P], f8, kind="ExternalInput")
    wus_in = nc.dram_tensor("wus_in", [M_ISH, P, KC2, 2, P], f8, kind="ExternalInput")
    wds_in = nc.dram_tensor("wds_in", [P, HB, IC2, 2, 512], f8, kind="ExternalInput")
    xq_in = nc.dram_tensor("xq_in", [P, KC2, 2, TOWN], f8, kind="ExternalInput")

    out_own = nc.dram_tensor("out_own", [TOWN, H], f32, kind="ExternalOutput")

    send_buf = nc.dram_tensor("send_buf", [SEND_ROWS, H], bf16)
    recv_buf = nc.dram_tensor("recv_buf", [SEND_ROWS, H], bf16)

    # segment -> matmul column blocks (<=512 for one PSUM bank)
    seg_blocks = []
    for s in range(NSEG):
        blocks, b0 = [], 0
        while b0 < SEG_SIZES[s]:
            bn = min(512, SEG_SIZES[s] - b0)
            blocks.append((b0, bn))
            b0 += bn
        seg_blocks.append(blocks)

    with tile.TileContext(nc) as tc:
      with (
          tc.tile_pool(name="const", bufs=1) as constp,
          tc.tile_pool(name="big", bufs=1) as bigp,
          tc.tile_pool(name="io", bufs=2) as iop,
      ):
        for _rep in range(n_reps):
            shared_tok = bigp.tile([P, TOWN // P, H], bf16, name="shared_tok",
                                   tag="shared_tok")
            # shared-expert inputs hoisted: prefetch during the routed phase
            xq = bigp.tile([P, KC2, 2, TOWN], f8, name="xq", tag="xq")
            h8 = bigp.tile([P, IC2, 2, TOWN], f8, name="h8", tag="h8")
            if "shared" not in SKIP_PHASES:
                nc.sync.dma_start(xq[:], xq_in[:])

            wts = []
            sidx = []
            for st in range(N_TILES):
                w_t = constp.tile([P, 1], f32, name=f"w_t{st}", tag=f"w_t{st}")
                nc.sync.dma_start(w_t[:], wslot[st][:, None])
                wts.append(w_t)
                si_t = constp.tile([P, 1], i32, name=f"si_t{st}", tag=f"si_t{st}")
                nc.sync.dma_start(si_t[:], send_pos[st][:, None])
                sidx.append(si_t)

            # ---------------- routed experts: NSEG segments ----------------
            if "routed" not in SKIP_PHASES:
              with (
                  tc.tile_pool(name="rtbig", bufs=1) as rtbig,
                  tc.tile_pool(name="rtw", bufs=2) as rtw,
                  tc.tile_pool(name="rtwork", bufs=2) as work,
              ):
                  xgT = rtbig.tile([P, KC_H, CAP], bf16, name="xgT", tag="xgT")
                  for kc in range(KC_H):
                      nc.sync.dma_start(xgT[:, kc, :], xg_in[kc])
                  h = rtbig.tile([P, M_I, CAP], bf16, name="h", tag="h")

                  up_ps = tc.tile_pool(name="upps", bufs=1, space="PSUM")
                  psp = up_ps.__enter__()
                  for s in range(NSEG):
                      for m in range(M_I):
                          wg_t = rtw.tile([P, KC_H, P], bf16, name="wg_t", tag="wg_t")
                          wu_t = rtw.tile([P, KC_H, P], bf16, name="wu_t", tag="wu_t")
                          if "wdma" not in SKIP_PHASES:
                              nc.sync.dma_start(wg_t[:], wg_in[s, m])
                              nc.sync.dma_start(wu_t[:], wu_in[s, m])
                          for (b0, bn) in seg_blocks[s]:
                              if "mm" in SKIP_PHASES:
                                  continue
                              a0 = SEG_OFF[s] + b0
                              psg2 = psp.tile([P, 512], f32, name="psg2", tag="psg", bufs=2)
                              psu2 = psp.tile([P, 512], f32, name="psu2", tag="psu", bufs=2)
                              for kc in range(KC_H):
                                  nc.tensor.matmul(psg2[:, :bn], wg_t[:, kc, :],
                                                   xgT[:, kc, a0:a0 + bn],
                                                   start=(kc == 0), stop=(kc == KC_H - 1))
                              for kc in range(KC_H):
                                  nc.tensor.matmul(psu2[:, :bn], wu_t[:, kc, :],
                                                   xgT[:, kc, a0:a0 + bn],
                                                   start=(kc == 0), stop=(kc == KC_H - 1))
                              sg2 = work.tile([P, 512], bf16, name="sg2", tag="sg2")
                              nc.scalar.activation(sg2[:, :bn], psg2[:, :bn], AF.Silu)
                              nc.vector.tensor_mul(h[:, m, a0:a0 + bn], sg2[:, :bn], psu2[:, :bn])
                  up_ps.__exit__(None, None, None)

                  # down projection by tile groups; scale; scatter to send_buf
                  for (g0, g1) in TILE_GROUPS:
                      dn_ps = tc.tile_pool(name=f"dnps{g0}", bufs=1, space="PSUM")
                      dpsp = dn_ps.__enter__()
                      y_tok = {t: rtbig.tile([P, H], bf16, name=f"y_tok{t}",
                                             tag=f"y_tok{t - g0}")
                               for t in range(g0, g1)}
                      segs_here = sorted({s for t in range(g0, g1)
                                          for (s, _, _) in _tile_subranges(t)})
                      for hb in range(HB):
                          ps_d = {t: dpsp.tile([P, 512], f32, name=f"ps_d{t}",
                                               tag=f"ps_d{t - g0}")
                                  for t in range(g0, g1)}
                          for s in segs_here:
                              for ic in range(M_I):
                                  wd_t = rtw.tile([P, 512], bf16, name="wd_t", tag="wd_t", bufs=4)
                                  if "wdma" not in SKIP_PHASES:
                                      nc.sync.dma_start(wd_t[:], wd_in[s, ic][:, hb * 512:(hb + 1) * 512])
                                  if "mm" in SKIP_PHASES:
                                      continue
                                  for t in range(g0, g1):
                                      for (ss, a, b) in _tile_subranges(t):
                                          if ss != s:
                                              continue
                                          p0 = a - t * P
                                          nc.tensor.matmul(
                                              ps_d[t][p0:p0 + (b - a), :],
                                              h[:, ic, a:b], wd_t[:],
                                              start=(ic == 0), stop=(ic == M_I - 1))
                          for t in range(g0, g1):
                              nc.vector.tensor_scalar_mul(
                                  y_tok[t][:, hb * 512:(hb + 1) * 512],
                                  ps_d[t][:], wts[t][:, :1])
                      for t in range(g0, g1):
                          if "scatter" in SKIP_PHASES:
                              continue
                          nc.gpsimd.indirect_dma_start(
                              out=send_buf[:, :], in_=y_tok[t][:],
                              out_offset=bass.IndirectOffsetOnAxis(ap=sidx[t][:, :1], axis=0),
                              in_offset=None,
                              bounds_check=SEND_ROWS - 1,
                              oob_is_err=False)
                      dn_ps.__exit__(None, None, None)

            # ---------------- all-to-all combine ----------------
            if "a2a" not in SKIP_PHASES:
              nc.gpsimd.collective_compute(
                "AllToAll",
                mybir.AluOpType.bypass,
                replica_groups=[list(range(NC))],
                ins=[send_buf[:, :].opt()],
                outs=[recv_buf[:, :].opt()],
              )

            # ---------------- shared expert (own 512 tokens, fp8 DoubleRow) --
            if "shared" in SKIP_PHASES:
                nc.vector.memset(shared_tok[:], 0.0)
                shb = None
            else:
              with (
                  tc.tile_pool(name="shbig", bufs=1) as shbig,
                  tc.tile_pool(name="shw", bufs=2) as shw,
                  tc.tile_pool(name="shps", bufs=1, space="PSUM") as psp,
              ):
                  # all down-proj weights resident (32KB/partition in fp8)
                  wds_t = shbig.tile([P, HB, IC2, 2, 512], f8, name="wds_t", tag="wds_t")
                  nc.sync.dma_start(wds_t[:], wds_in[:])

                  for m in range(M_ISH):
                      wgs_t = shw.tile([P, KC2, 2, P], f8, name="wgs_t", tag="wgs_t", bufs=4)
                      wus_t = shw.tile([P, KC2, 2, P], f8, name="wus_t", tag="wus_t", bufs=4)
                      nc.sync.dma_start(wgs_t[:], wgs_in[m])
                      nc.sync.dma_start(wus_t[:], wus_in[m])
                      psg = psp.tile([P, TOWN], f32, name="psg", tag="psg", bufs=2)
                      psu = psp.tile([P, TOWN], f32, name="psu", tag="psu", bufs=2)
                      for kc in range(KC2):
                          nc.tensor.matmul(psg[:], wgs_t[:, kc, :, :], xq[:, kc, :, :],
                                           start=(kc == 0), stop=(kc == KC2 - 1),
                                           perf_mode=DR)
                      for kc in range(KC2):
                          nc.tensor.matmul(psu[:], wus_t[:, kc, :, :], xq[:, kc, :, :],
                                           start=(kc == 0), stop=(kc == KC2 - 1),
                                           perf_mode=DR)
                      sg = shw.tile([P, TOWN], bf16, name="sg", tag="sg")
                      su = shw.tile([P, TOWN], bf16, name="su", tag="su")
                      nc.scalar.activation(sg[:], psg[:], AF.Silu, scale=1.0 / SW)
                      nc.scalar.activation(su[:], psu[:], AF.Copy, scale=SH / SW)
                      nc.vector.tensor_mul(h8[:, m // 2, m % 2, :], sg[:], su[:])

                  # shared down-projection, token-tile-major so combine overlaps
                  with tc.tile_pool(name="cmb", bufs=2) as cmb:
                      for tt in range(TOWN // P):
                          i1 = iop.tile([P, 1], i32, name="i1", tag="i1")
                          i2 = iop.tile([P, 1], i32, name="i2", tag="i2")
                          nc.sync.dma_start(i1[:], recv_idx[0, tt][:, None])
                          nc.sync.dma_start(i2[:], recv_idx[1, tt][:, None])
                          g1 = cmb.tile([P, H], bf16, name="g1", tag="g1")
                          g2 = cmb.tile([P, H], bf16, name="g2", tag="g2")
                          if "a2a" not in SKIP_PHASES:
                              nc.gpsimd.indirect_dma_start(
                                  out=g1[:], out_offset=None, in_=recv_buf[:, :],
                                  in_offset=bass.IndirectOffsetOnAxis(ap=i1[:, :1], axis=0))
                              nc.gpsimd.indirect_dma_start(
                                  out=g2[:], out_offset=None, in_=recv_buf[:, :],
                                  in_offset=bass.IndirectOffsetOnAxis(ap=i2[:, :1], axis=0))
                          else:
                              nc.vector.memset(g1[:], 0.0)
                              nc.vector.memset(g2[:], 0.0)
                          acc = cmb.tile([P, H], f32, name="acc", tag="acc")
                          for hb in range(HB):
                              ps_sh = psp.tile([P, 512], f32, name="ps_sh",
                                               tag="ps_sh", bufs=2)
                              for ic in range(IC2):
                                  nc.tensor.matmul(ps_sh[:], h8[:, ic, :, tt * P:(tt + 1) * P],
                                                   wds_t[:, hb, ic, :, :],
                                                   start=(ic == 0), stop=(ic == IC2 - 1),
                                                   perf_mode=DR)
                              nc.scalar.activation(
                                  shared_tok[:, tt, hb * 512:(hb + 1) * 512],
                                  ps_sh[:], AF.Copy, scale=RATIO / (SH * SW))
                          nc.vector.tensor_add(acc[:], g1[:], g2[:])
                          nc.vector.tensor_add(acc[:], acc[:], shared_tok[:, tt, :])
                          nc.sync.dma_start(out_own[tt * P:(tt + 1) * P, :], acc[:])

            if "shared" in SKIP_PHASES:
              # fallback combine without shared overlap (debug only)
              with tc.tile_pool(name="cmb", bufs=2) as cmb:
                for tt in range(TOWN // P):
                    i1 = iop.tile([P, 1], i32, name="i1", tag="i1")
                    i2 = iop.tile([P, 1], i32, name="i2", tag="i2")
                    nc.sync.dma_start(i1[:], recv_idx[0, tt][:, None])
                    nc.sync.dma_start(i2[:], recv_idx[1, tt][:, None])
                    g1 = cmb.tile([P, H], bf16, name="g1", tag="g1")
                    g2 = cmb.tile([P, H], bf16, name="g2", tag="g2")
                    nc.gpsimd.indirect_dma_start(
                        out=g1[:], out_offset=None, in_=recv_buf[:, :],
                        in_offset=bass.IndirectOffsetOnAxis(ap=i1[:, :1], axis=0))
                    nc.gpsimd.indirect_dma_start(
                        out=g2[:], out_offset=None, in_=recv_buf[:, :],
                        in_offset=bass.IndirectOffsetOnAxis(ap=i2[:, :1], axis=0))
                    acc = cmb.tile([P, H], f32, name="acc", tag="acc")
                    nc.vector.tensor_add(acc[:], g1[:], g2[:])
                    nc.vector.tensor_add(acc[:], acc[:], shared_tok[:, tt, :])
                    nc.sync.dma_start(out_own[tt * P:(tt + 1) * P, :], acc[:])

    nc.finalize()
    return nc


def prepare_in_maps(x, centroids, gate_bias, wg_s, wu_s, wd_s, wg, wu, wd):
    x = np.ascontiguousarray(np.asarray(x, dtype=np.float32))
    wg = np.asarray(wg, dtype=np.float32)
    wu = np.asarray(wu, dtype=np.float32)
    wd = np.asarray(wd, dtype=np.float32)

    top_idx, weights = _host_routing(x, centroids, gate_bias)

    # expert token lists in token order
    lists = [[] for _ in range(E)]
    wvals = [[] for _ in range(E)]
    for t in range(T):
        for k in range(TOPK):
            e = int(top_idx[t, k])
            lists[e].append(t)
            wvals[e].append(weights[t, k])
    counts = [len(l) for l in lists]
    cores = _assign_segments(counts)
    cores, lists, wvals = _stride_split_experts(cores, lists, wvals)
    cores = _optimize_pairing(cores, lists)

    # per-core slot tables
    tok_ids = np.full((NC, N_TILES, P), DUMMY_TOK, dtype=np.int32)
    wslot = np.zeros((NC, N_TILES, P), dtype=np.float32)
    piece_expert = np.zeros((NC, NSEG), dtype=np.int64)
    for c in range(NC):
        loc = 0
        for pi, (e, off, cnt, sz) in enumerate(cores[c]):
            piece_expert[c, pi] = e
            pts = [(lists[e][off + j], wvals[e][off + j]) for j in range(cnt)]
            pts.sort(key=lambda tw: (tw[0] // TOWN, tw[0]))
            for j, (t, w) in enumerate(pts):
                tok_ids[c, (loc + j) // P, (loc + j) % P] = t
                wslot[c, (loc + j) // P, (loc + j) % P] = w
            loc += sz

    # send positions / recv indices
    cnt_co = np.zeros((NC, NC), dtype=np.int64)
    contrib = [[] for _ in range(T)]  # (core, pos) per contribution
    for c in range(NC):
        for loc in range(CAP):
            t = int(tok_ids[c, loc // P, loc % P])
            if t == DUMMY_TOK:
                continue
            o = t // TOWN
            pos = cnt_co[c, o]
            cnt_co[c, o] += 1
            contrib[t].append((c, int(pos)))
    SCAP = int(((cnt_co.max() + 15) // 16) * 16)
    # destination row = owner * SCAP + pos
    send_pos_arr = np.full((NC, N_TILES, P), BIG, dtype=np.int32)
    cnt_co2 = np.zeros((NC, NC), dtype=np.int64)
    for c in range(NC):
        for loc in range(CAP):
            t = int(tok_ids[c, loc // P, loc % P])
            if t == DUMMY_TOK:
                continue
            o = t // TOWN
            pos = cnt_co2[c, o]
            cnt_co2[c, o] += 1
            send_pos_arr[c, loc // P, loc % P] = o * SCAP + pos

    recv_idx = np.zeros((NC, 2, TOWN // P, P), dtype=np.int32)
    for t in range(T):
        o = t // TOWN
        tl = t % TOWN
        assert len(contrib[t]) == 2, (t, contrib[t])
        for k, (c, pos) in enumerate(contrib[t]):
            recv_idx[o, k, tl // P, tl % P] = c * SCAP + pos

    # weight tensors, matmul-ready tiling
    def tile_up(w2d, mm):  # [H, mm*128] -> [mm, 128, KC_H, 128]
        return np.ascontiguousarray(
            w2d.reshape(KC_H, P, mm, P).transpose(2, 1, 0, 3))

    def tile_dn(w2d, mm):  # [mm*128, H] -> [mm, 128, H]
        return np.ascontiguousarray(w2d.reshape(mm, P, H))

    wg_t = np.zeros((NC, NSEG, M_I, P, KC_H, P), dtype=BF16)
    wu_t = np.zeros((NC, NSEG, M_I, P, KC_H, P), dtype=BF16)
    wd_t = np.zeros((NC, NSEG, M_I, P, H), dtype=BF16)
    done = {}
    for c in range(NC):
        for pi, (e, off, cnt, sz) in enumerate(cores[c]):
            if cnt == 0:
                continue
            if e not in done:
                done[e] = (tile_up(wg[e], M_I).astype(BF16),
                           tile_up(wu[e], M_I).astype(BF16),
                           tile_dn(wd[e], M_I).astype(BF16))
            wg_t[c, pi], wu_t[c, pi], wd_t[c, pi] = done[e]

    # shared-expert fp8 tensors (scaled), DoubleRow layouts
    def tile_up8(w2d):  # [H, ISH] -> [M_ISH, P, KC2, 2, P]
        w = (np.asarray(w2d, np.float32) * SW).reshape(KC2, 2, P, M_ISH, P)
        return np.ascontiguousarray(w.transpose(3, 2, 0, 1, 4)).astype(F8)

    def tile_dn8(w2d):  # [ISH, H] -> [P, HB, IC2, 2, 512]
        w = (np.asarray(w2d, np.float32) * SW).reshape(IC2, 2, P, HB, 512)
        return np.ascontiguousarray(w.transpose(2, 3, 0, 1, 4)).astype(F8)

    wgs_t = tile_up8(wg_s)
    wus_t = tile_up8(wu_s)
    wds_t = tile_dn8(wd_s)

    x_flat = x.reshape(T, H)
    x_pad = np.vstack([x_flat, np.zeros((1, H), np.float32)])

    in_maps = []
    for c in range(NC):
        xo = x_flat[c * TOWN:(c + 1) * TOWN]  # [TOWN, H]
        # [P, KC2, 2, TOWN] with h = kc2*256 + j*128 + p
        xq = np.ascontiguousarray(
            xo.T.reshape(KC2, 2, P, TOWN).transpose(2, 0, 1, 3)).astype(F8)
        xg_c = np.ascontiguousarray(
            x_pad[tok_ids[c].reshape(-1)].T.reshape(KC_H, P, CAP)).astype(BF16)
        in_maps.append({
            "xg_in": xg_c,
            "wslot": wslot[c],
            "send_pos": send_pos_arr[c],
            "recv_idx": recv_idx[c],
            "wg_in": wg_t[c],
            "wu_in": wu_t[c],
            "wd_in": wd_t[c],
            "wgs_in": wgs_t,
            "wus_in": wus_t,
            "wds_in": wds_t,
            "xq_in": xq,
        })

    return in_maps, SCAP


def get_program(scap, n_reps=1):
    key = ("moe", scap, n_reps)
    if key not in _COMPILED:
        _COMPILED[key] = _build_program(scap, n_reps)
    return _COMPILED[key]


_RUNNER = {}


def _build_runner(nc, n_cores=NC):
    """Build a reusable PJRT executable for the finalized Bass program.
    Mirrors concourse.bass2jax.run_bass_via_pjrt but without output donation,
    so the jitted callable can be invoked repeatedly and its HLO is stable
    across processes (persistent-cache friendly)."""
    import jax
    import concourse.mybir as mybir
    from concourse import bass2jax as b2j
    from jax.experimental.shard_map import shard_map
    from jax.sharding import Mesh, PartitionSpec, NamedSharding

    b2j.install_neuronx_cc_hook()
    partition_name = nc.partition_id_tensor.name if nc.partition_id_tensor else None
    in_names, out_names, out_avals, zero_outs = [], [], [], []
    for alloc in nc.m.functions[0].allocations:
        if not isinstance(alloc, mybir.MemoryLocationSet):
            continue
        name = alloc.memorylocations[0].name
        if alloc.kind == "ExternalInput":
            if name != partition_name:
                in_names.append(name)
        elif alloc.kind == "ExternalOutput":
            shape = tuple(alloc.tensor_shape)
            dtype = mybir.dt.np(alloc.dtype)
            out_avals.append(jax.core.ShapedArray(shape, dtype))
            out_names.append(name)
            zero_outs.append(np.zeros(shape, dtype))
    n_params = len(in_names)
    all_in_names = in_names + out_names
    if partition_name is not None:
        all_in_names = all_in_names + [partition_name]

    def _body(*args):
        operands = list(args)
        if partition_name is not None:
            operands.append(b2j.partition_id_tensor())
        outs = b2j._bass_exec_p.bind(
            *operands,
            out_avals=tuple(out_avals),
            in_names=tuple(all_in_names),
            out_names=tuple(out_names),
            lowering_input_output_aliases=(),
            sim_require_finite=True,
            sim_require_nnan=True,
            nc=nc,
        )
        return tuple(outs)

    devices = jax.devices()[:n_cores]
    mesh = Mesh(np.asarray(devices), ("core",))
    spec = PartitionSpec("core")
    sharded = jax.jit(
        shard_map(_body, mesh=mesh, in_specs=(spec,) * (n_params + len(out_names)),
                  out_specs=(spec,) * len(out_names), check_rep=False),
        keep_unused=True,
    )
    sh = NamedSharding(mesh, spec)

    def run(in_maps):
        concat_in = [
            np.concatenate([np.asarray(in_maps[c][nm]) for c in range(n_cores)], axis=0)
            for nm in in_names
        ]
        concat_zeros = [np.zeros((n_cores * z.shape[0], *z.shape[1:]), z.dtype)
                        for z in zero_outs]
        dev_in = [jax.device_put(a, sh) for a in concat_in]
        dev_zero = [jax.device_put(a, sh) for a in concat_zeros]
        out = sharded(*dev_in, *dev_zero)
        jax.block_until_ready(out)
        return ({nm: np.asarray(out[i]) for i, nm in enumerate(out_names)},
                (sharded, dev_in, dev_zero))

    return run


def kernel(x, centroids, gate_bias, wg_s, wu_s, wd_s, wg, wu, wd):
    _enable_jax_cache()
    in_maps, scap = prepare_in_maps(x, centroids, gate_bias, wg_s, wu_s, wd_s, wg, wu, wd)
    nc = get_program(scap)
    key = ("run", scap)
    if key not in _RUNNER:
        _RUNNER[key] = _build_runner(nc)
    outs, _ = _RUNNER[key](in_maps)
    out = outs["out_own"].reshape(NC, TOWN, H)
    return np.ascontiguousarray(out.reshape(B, S, H))
